# revision 1
# baseline (speedup 1.0000x reference)
"""Tensor-parallel causal multi-head attention for 8 TRN2 NeuronCores.

Problem: B=2, T=2048, HIDDEN=2048, 16 heads x 128 head_dim, causal, RoPE.
Sharding: 2 heads per core (tensor parallel). Each core computes its QKV
projections, RoPE, causal attention, and a partial output projection over
its 256 hidden features; the host sums the 8 partial outputs.

Device compute dtype: bf16 matmuls with f32 PSUM accumulation; softmax in
f32 (no max-subtraction needed: |scores/sqrt(d)| < ~8 for this data scale).

Layouts (per core):
  xt    [2048 c, 4096 t]  bf16   (x transposed; contraction dim on partitions)
  wqt/wkt/wvt [2048 c, 256 d] bf16 (per-core head-slice of weights, transposed)
  wot   [256 c, 2048 d]  bf16   (per-core row-slice of wo.T)
  cos2/sin2 [2048 t, 128] f32   (freqs duplicated across the 2 local heads)
  out   [4096 t, 2048 d]  f32   partial output (host sums over cores)
"""

import numpy as np
import ml_dtypes
from contextlib import ExitStack

import concourse.bass as bass
import concourse.mybir as mybir
import concourse.tile as tile
from concourse import bacc
from concourse.bass_utils import run_bass_kernel_spmd
from concourse.masks import make_identity

F32 = mybir.dt.float32
BF16 = mybir.dt.bfloat16

NCORES = 8
B, T, C = 2, 2048, 2048
TT = B * T              # 4096 flattened rows
NH, D = 16, 128         # global heads, head dim
HL = NH // NCORES       # 2 local heads
DH = HL * D             # 256 local head features
NE = 8                  # t-eighths of 512 rows
ET = TT // NE           # 512 rows per eighth
CT = C // 128           # 16 contraction tiles
SCALE = 1.0 / float(np.sqrt(D))

_CACHE: dict = {}


def _build(T=T, B=B, num_devices=NCORES, debug_outs=False, repeat=1,
           small_out=False, stop_after=None, ablate=(), attn_cfg="pair"):
    TT = B * T
    NE = TT // 512
    ET = 512
    nc = bacc.Bacc("TRN2", target_bir_lowering=False, debug=False,
                   num_devices=num_devices)
    xt = nc.dram_tensor("xt", [C, TT], BF16, kind="ExternalInput").ap()
    wqt = nc.dram_tensor("wqt", [C, DH], BF16, kind="ExternalInput").ap()
    wkt = nc.dram_tensor("wkt", [C, DH], BF16, kind="ExternalInput").ap()
    wvt = nc.dram_tensor("wvt", [C, DH], BF16, kind="ExternalInput").ap()
    wot = nc.dram_tensor("wot", [DH, C], BF16, kind="ExternalInput").ap()
    cos2 = nc.dram_tensor("cos2", [T, 2 * (D // 2)], F32, kind="ExternalInput").ap()
    sin2 = nc.dram_tensor("sin2", [T, 2 * (D // 2)], F32, kind="ExternalInput").ap()
    _odt = F32 if "f32out" in ablate else BF16
    out = nc.dram_tensor("out", [128 if small_out else TT, C], _odt,
                         kind="ExternalOutput").ap()
    if debug_outs:
        dbg_q = nc.dram_tensor("dbg_q", [HL * 128, TT], F32, kind="ExternalOutput").ap()
        dbg_k = nc.dram_tensor("dbg_k", [HL * 128, TT], F32, kind="ExternalOutput").ap()
        dbg_v = nc.dram_tensor("dbg_v", [128, (TT // 128) * DH], F32, kind="ExternalOutput").ap()

    with ExitStack() as ctx:
        tc = ctx.enter_context(tile.TileContext(nc))
        # ---- persistent tiles -------------------------------------------
        gp = ctx.enter_context(tc.tile_pool(name="glob", bufs=1))
        # wqk packs [wq_c | wk_c] per c-tile so Q and K come from ONE
        # N=512 matmul (one PSUM accumulation group per bank).
        wqk_sb = gp.tile([128, CT * 2 * DH], BF16)   # [128, 8192]
        wv_sb = gp.tile([128, CT * DH], BF16)
        wo_sb = gp.tile([128, HL * C], BF16)    # [128, 4096]
        qk_view = wqk_sb[:].rearrange("p (k d) -> p k d", d=2 * DH)
        nc.sync.dma_start(qk_view[:, :, 0:DH],
                          wqt.rearrange("(k p) d -> p k d", p=128))
        nc.sync.dma_start(qk_view[:, :, DH:2 * DH],
                          wkt.rearrange("(k p) d -> p k d", p=128))
        for dst, src_ap, nd in ((wv_sb, wvt, DH), (wo_sb, wot, C)):
            nc.sync.dma_start(
                dst[:].rearrange("p (k d) -> p k d", d=nd),
                src_ap.rearrange("(k p) d -> p k d", p=128))

        v_all = gp.tile([128, (TT // 128) * DH], BF16)   # [128, 8192]
        qT = [gp.tile([128, TT], BF16, tag=f"qT{h}", name=f"qT{h}") for h in range(HL)]
        kT = [gp.tile([128, TT], BF16, tag=f"kT{h}", name=f"kT{h}") for h in range(HL)]

        ident = gp.tile([128, 128], BF16)
        make_identity(nc, ident[:])
        ones_col = gp.tile([128, 1], BF16)
        nc.vector.memset(ones_col[:], 1.0)
        ones_row = gp.tile([1, 128], F32)
        nc.vector.memset(ones_row[:], 1.0)

        # static causal masks for the 4 diagonal block offsets (f32-exact
        # iota, stored bf16 0/1): mask_k keeps [x, y] iff x <= y - 128k
        pairmasks = []
        mtmp = gp.tile([128, 512], F32)
        for m in range(2):
            pm = gp.tile([128, 1024], BF16, tag=f"pmask{m}", name=f"pmask{m}")
            for half in range(2):
                k = 2 * m + half
                nc.vector.memset(mtmp[:], 1.0)
                nc.gpsimd.affine_select(
                    out=mtmp[:], in_=mtmp[:],
                    compare_op=mybir.AluOpType.is_ge, fill=0.0,
                    base=-128 * k, pattern=[[1, 512]], channel_multiplier=-1,
                )
                nc.vector.tensor_copy(pm[:, half * 512:(half + 1) * 512], mtmp[:])
            pairmasks.append(pm)

        # ---- phase 1: QKV projections + RoPE + transposes ---------------
        for _rep in range(repeat):
         with ExitStack() as p1:
            xp = p1.enter_context(tc.tile_pool(name="xin", bufs=6))
            tp = p1.enter_context(tc.tile_pool(name="trig", bufs=2))
            sp = p1.enter_context(tc.tile_pool(name="stage", bufs=3))
            rp = p1.enter_context(tc.tile_pool(name="rtmp", bufs=3))
            pqk = p1.enter_context(tc.tile_pool(name="pqk", bufs=4, space="PSUM"))
            pv = p1.enter_context(tc.tile_pool(name="pv", bufs=2, space="PSUM"))
            pt = p1.enter_context(tc.tile_pool(name="ptr", bufs=2, space="PSUM"))

            for e in range(NE):
                t0 = e * ET  # global row offset of this eighth
                # per-eighth trig tiles [128, 4 x 128] (tt-major)
                ct_sb = tp.tile([128, 4 * 128], F32, tag="cos")
                st_sb = tp.tile([128, 4 * 128], F32, tag="sin")
                trow = (t0 % T)
                nc.sync.dma_start(
                    ct_sb[:].rearrange("p (tt d) -> p tt d", d=128),
                    cos2[trow:trow + ET, :].rearrange("(tt p) d -> p tt d", p=128))
                nc.sync.dma_start(
                    st_sb[:].rearrange("p (tt d) -> p tt d", d=128),
                    sin2[trow:trow + ET, :].rearrange("(tt p) d -> p tt d", p=128))

                pQK = [pqk.tile([128, 512], F32, tag="pqk", name=f"pQK{_}") for _ in range(4)]
                pVT = [pv.tile([128, 512], F32, tag="pv", name=f"pVT{_}") for _ in range(2)]

                for c in range(CT):
                    xc = xp.tile([128, ET], BF16, tag="xc")
                    dma_eng = nc.sync if c % 2 == 0 else nc.scalar
                    dma_eng.dma_start(
                        xc[:], xt[c * 128:(c + 1) * 128, t0:t0 + ET])
                    st = (c == 0)
                    sp_ = (c == CT - 1)
                    for tt in range(4):
                        nc.tensor.matmul(
                            pQK[tt][:], xc[:, tt * 128:(tt + 1) * 128],
                            wqk_sb[:, c * 2 * DH:(c + 1) * 2 * DH],
                            start=st, stop=sp_)
                    for dt in range(2):
                        nc.tensor.matmul(
                            pVT[dt][:], wv_sb[:, c * DH + dt * 128: c * DH + (dt + 1) * 128],
                            xc[:], start=st, stop=sp_)

                # V^T: PSUM -> bf16 SBUF, then PE-transpose into v_all [t, d]
                for dt in range(2):
                    vts = sp.tile([128, ET], BF16, tag=f"vts{dt}", name=f"vts{dt}")
                    nc.scalar.copy(vts[:], pVT[dt][:])
                    for tt in range(4):
                        g = (t0 // 128) + tt
                        pb = pt.tile([128, 128], BF16, tag="ptr", name="pbv")
                        nc.tensor.transpose(
                            pb[:], vts[:, tt * 128:(tt + 1) * 128], ident[:])
                        nc.scalar.copy(
                            v_all[:, g * DH + dt * 128: g * DH + (dt + 1) * 128],
                            pb[:])

                # Q/K: PSUM -> f32 staging
                qs = sp.tile([128, 4 * DH], F32, tag="qs")
                ks = sp.tile([128, 4 * DH], F32, tag="ks")
                for tt in range(4):
                    nc.scalar.copy(qs[:, tt * DH:(tt + 1) * DH],
                                   pQK[tt][:, 0:256])
                    nc.scalar.copy(ks[:, tt * DH:(tt + 1) * DH],
                                   pQK[tt][:, 256:512])

                # RoPE in [t, d] layout; pairs along free dim.
                qr = rp.tile([128, 4 * DH], BF16, tag="qr")
                kr = rp.tile([128, 4 * DH], BF16, tag="kr")
                tm1 = rp.tile([128, 4 * DH], F32, tag="tm1")
                tm2 = rp.tile([128, 4 * DH], F32, tag="tm2")
                cv = ct_sb[:].rearrange("p (tt h j) -> p tt h j", tt=4, h=HL)
                sv = st_sb[:].rearrange("p (tt h j) -> p tt h j", tt=4, h=HL)
                for src, dst in ((qs, qr), (ks, kr)):
                    s4 = src[:].rearrange(
                        "p (tt h j two) -> p tt h j two", tt=4, h=HL, two=2)
                    d4 = dst[:].rearrange(
                        "p (tt h j two) -> p tt h j two", tt=4, h=HL, two=2)
                    t14 = tm1[:].rearrange(
                        "p (tt h j two) -> p tt h j two", tt=4, h=HL, two=2)
                    t24 = tm2[:].rearrange(
                        "p (tt h j two) -> p tt h j two", tt=4, h=HL, two=2)
                    xe, xo = s4[:, :, :, :, 0], s4[:, :, :, :, 1]
                    nc.vector.tensor_mul(t14[:, :, :, :, 0], xe, cv)
                    nc.vector.tensor_mul(t24[:, :, :, :, 0], xo, sv)
                    nc.vector.tensor_sub(d4[:, :, :, :, 0],
                                         t14[:, :, :, :, 0], t24[:, :, :, :, 0])
                    nc.vector.tensor_mul(t14[:, :, :, :, 1], xe, sv)
                    nc.vector.tensor_mul(t24[:, :, :, :, 1], xo, cv)
                    nc.vector.tensor_add(d4[:, :, :, :, 1],
                                         t14[:, :, :, :, 1], t24[:, :, :, :, 1])

                # transpose Q/K blocks [128t, 128d] -> [128d, 128t]
                for src, dstl in ((qr, qT), (kr, kT)):
                    for tt in range(4):
                        for h in range(HL):
                            pb = pt.tile([128, 128], BF16, tag="ptr")
                            nc.tensor.transpose(
                                pb[:], src[:, tt * DH + h * 128: tt * DH + (h + 1) * 128],
                                ident[:])
                            nc.scalar.copy(
                                dstl[h][:, t0 + tt * 128: t0 + (tt + 1) * 128],
                                pb[:])

         if debug_outs:
             with tc.tile_pool(name="dbgp", bufs=1) as dbgp:
                 dq = dbgp.tile([128, TT], F32, name="dq")
                 for h in range(HL):
                     nc.vector.tensor_copy(dq[:], qT[h][:])
                     nc.sync.dma_start(dbg_q[h * 128:(h + 1) * 128, :], dq[:])
                     nc.vector.tensor_copy(dq[:], kT[h][:])
                     nc.sync.dma_start(dbg_k[h * 128:(h + 1) * 128, :], dq[:])
                 dv = dbgp.tile([128, (TT // 128) * DH], F32, name="dv")
                 nc.vector.tensor_copy(dv[:], v_all[:])
                 nc.sync.dma_start(dbg_v[:], dv[:])

         if stop_after == "qkv":
             with tc.tile_pool(name="dump", bufs=1) as dump:
                 dt_ = dump.tile([128, TT], F32, name="dt_")
                 nc.vector.tensor_copy(dt_[:], qT[0][:])
                 nc.vector.tensor_add(dt_[:], dt_[:], kT[1][:])
                 nc.vector.tensor_add(dt_[:], dt_[:], v_all[:, 0:TT])
                 nc.sync.dma_start(out[0:128, 0:C], dt_[:, 0:C])
                 nc.vector.tensor_copy(dt_[:], qT[1][:])
                 nc.vector.tensor_add(dt_[:], dt_[:], kT[0][:])
                 nc.sync.dma_start(out[0:128, 0:C], dt_[:, 0:C])
             continue

         # ---- phase 2: attention + output projection ---------------------
         with ExitStack() as p2:
             ptp = p2.enter_context(tc.tile_pool(name="ptile", bufs=16))
             atp = p2.enter_context(tc.tile_pool(name="attnT", bufs=6))
             rdp = p2.enter_context(tc.tile_pool(name="rden", bufs=2))
             osp = p2.enter_context(tc.tile_pool(name="ost", bufs=4))
             _swb, _sob = {"pair": (2, 2), "u4": (4, 2), "u5": (5, 1)}[attn_cfg]
             psw = p2.enter_context(tc.tile_pool(name="psw", bufs=_swb, space="PSUM"))
             pso = p2.enter_context(tc.tile_pool(name="pso", bufs=_sob, space="PSUM"))
             psa = p2.enter_context(tc.tile_pool(name="psa", bufs=1, space="PSUM"))
             psd = p2.enter_context(tc.tile_pool(name="psd", bufs=1, space="PSUM"))

             for b in range(B):
                 for j in range(T // 512):   # q-chunks of 512 within the batch
                     q0 = b * T + j * 512
                     nkt = 4 * j + 4
                     attnT = []
                     for h in range(HL):
                         pA = psa.tile([128, 512], F32, tag="psa")
                         pDen = psd.tile([1, 512], F32, tag="psd")
                         # kt-tiles processed in PAIRS: scores land in the two
                         # banks of one [128,1024] PSUM tile; ONE exp covers
                         # both halves (halves the ACT op count + sem hops).
                         npair = nkt // 2
                         if attn_cfg != "pair":
                             for p_ in range(npair):
                                 ptile = ptp.tile([128, 1024], BF16, tag="ptile")
                                 for half in range(2):
                                     i = 2 * p_ + half
                                     g = b * (T // 128) + i
                                     pS1 = psw.tile([128, 512], F32, tag="psw1",
                                                    name="pS1")
                                     nc.tensor.matmul(
                                         pS1[:], kT[h][:, g * 128:(g + 1) * 128],
                                         qT[h][:, q0:q0 + 512],
                                         start=True, stop=True)
                                     pt_h = ptile[:, half * 512:(half + 1) * 512]
                                     nc.scalar.activation(
                                         pt_h, pS1[:],
                                         mybir.ActivationFunctionType.Exp,
                                         scale=SCALE)
                                     if i >= 4 * j:
                                         nc.vector.tensor_mul(
                                             pt_h, pt_h,
                                             pairmasks[(i - 4 * j) // 2][:, (i % 2) * 512:(i % 2) * 512 + 512])
                                     nc.tensor.matmul(
                                         pA[:], v_all[:, g * DH + h * 128: g * DH + (h + 1) * 128],
                                         pt_h, start=(i == 0), stop=(i == nkt - 1))
                                 pds = rdp.tile([128, 512], BF16, tag="pds")
                                 nc.vector.tensor_add(
                                     pds[:], ptile[:, 0:512], ptile[:, 512:1024])
                                 nc.tensor.matmul(
                                     pDen[:], ones_col[:], pds[:],
                                     start=(p_ == 0), stop=(p_ == npair - 1))
                         else:
                          for p_ in range(npair):
                             pS = psw.tile([128, 1024], F32, tag="psw")
                             ptile = ptp.tile([128, 1024], BF16, tag="ptile")
                             for half in range(2):
                                 i = 2 * p_ + half
                                 g = b * (T // 128) + i
                                 nc.tensor.matmul(
                                     pS[:, half * 512:(half + 1) * 512],
                                     kT[h][:, g * 128:(g + 1) * 128],
                                     qT[h][:, q0:q0 + 512], start=True, stop=True)
                             _fn = (mybir.ActivationFunctionType.Copy
                                    if "noexp" in ablate else
                                    mybir.ActivationFunctionType.Exp)
                             nc.scalar.activation(
                                 ptile[:], pS[:], _fn, scale=SCALE)
                             # diagonal pairs: one mask multiply over the pair
                             if 2 * p_ >= 4 * j and "nomask" not in ablate:
                                 nc.vector.tensor_mul(
                                     ptile[:], ptile[:],
                                     pairmasks[p_ - 2 * j][:])
                             for half in range(2):
                                 i = 2 * p_ + half
                                 g = b * (T // 128) + i
                                 pt_h = ptile[:, half * 512:(half + 1) * 512]
                                 if "nopv" not in ablate:
                                     nc.tensor.matmul(
                                         pA[:], v_all[:, g * DH + h * 128: g * DH + (h + 1) * 128],
                                         pt_h, start=(i == 0), stop=(i == nkt - 1))
                             if "noden" not in ablate:
                                 # pre-add the two halves (bf16) -> ONE den
                                 # matmul per pair
                                 pds = rdp.tile([128, 512], BF16, tag="pds")
                                 nc.vector.tensor_add(
                                     pds[:], ptile[:, 0:512], ptile[:, 512:1024])
                                 nc.tensor.matmul(
                                     pDen[:], ones_col[:], pds[:],
                                     start=(p_ == 0), stop=(p_ == npair - 1))
                         rden = rdp.tile([1, 512], F32, tag="rden")
                         if "nopv" in ablate:
                             nc.vector.memset(pA[:], 1.0)
                         _dsrc = pA[0:1, :] if "noden" in ablate else pDen[:]
                         nc.vector.reciprocal(rden[:], _dsrc)
                         pB = pso.tile([128, 512], F32, tag="pso")
                         nc.tensor.matmul(pB[:], ones_row[:], rden[:],
                                          start=True, stop=True)
                         bc = rdp.tile([128, 512], F32, tag="bc")
                         nc.vector.tensor_copy(bc[:], pB[:])
                         aT = atp.tile([128, 512], BF16, tag=f"aT{h}")
                         nc.vector.tensor_mul(aT[:], pA[:], bc[:])
                         attnT.append(aT)

                     if stop_after == "attn":
                         ost = osp.tile([128, C], F32, tag="ost", name="osta")
                         nc.vector.tensor_copy(ost[:, 0:512], attnT[0][:])
                         nc.vector.tensor_copy(ost[:, 512:1024], attnT[1][:])
                         nc.sync.dma_start(out[0:128, 0:1024], ost[:, 0:1024])
                         continue
                     for tt in range(4):
                         r0 = q0 + tt * 128
                         ost = osp.tile([128, C], _odt, tag="ost")
                         for oc in range(4):
                             pO = pso.tile([128, 512], F32, tag="pso")
                             for h in range(HL):
                                 nc.tensor.matmul(
                                     pO[:], attnT[h][:, tt * 128:(tt + 1) * 128],
                                     wo_sb[:, h * C + oc * 512: h * C + oc * 512 + 512],
                                     start=(h == 0), stop=(h == HL - 1))
                             nc.vector.tensor_copy(ost[:, oc * 512:(oc + 1) * 512],
                                                   pO[:])
                         if "nostore" in ablate:
                             if tt == 0:
                                 nc.scalar.dma_start(out[0:128, :], ost[:])
                         elif small_out:
                             nc.scalar.dma_start(out[0:128, :], ost[:])
                         else:
                             nc.scalar.dma_start(out[r0:r0 + 128, :], ost[:])

    nc.compile()
    return nc


def _get_nc():
    if "nc" not in _CACHE:
        _CACHE["nc"] = _build()
    return _CACHE["nc"]


def kernel(x, wq, wk, wv, wo, freqs_cos, freqs_sin, mask=None, **_unused):
    bf = ml_dtypes.bfloat16
    nc = _get_nc()

    x = np.asarray(x, dtype=np.float32)
    xt = np.ascontiguousarray(x.reshape(TT, C).T).astype(bf)
    cos2 = np.ascontiguousarray(
        np.tile(np.asarray(freqs_cos, np.float32), (1, HL)))
    sin2 = np.ascontiguousarray(
        np.tile(np.asarray(freqs_sin, np.float32), (1, HL)))

    in_maps = []
    for i in range(NCORES):
        sl = slice(DH * i, DH * (i + 1))
        in_maps.append({
            "xt": xt,
            "wqt": np.ascontiguousarray(np.asarray(wq, np.float32)[sl, :].T).astype(bf),
            "wkt": np.ascontiguousarray(np.asarray(wk, np.float32)[sl, :].T).astype(bf),
            "wvt": np.ascontiguousarray(np.asarray(wv, np.float32)[sl, :].T).astype(bf),
            "wot": np.ascontiguousarray(np.asarray(wo, np.float32)[:, sl].T).astype(bf),
            "cos2": cos2,
            "sin2": sin2,
        })

    res = run_bass_kernel_spmd(nc, in_maps, core_ids=list(range(NCORES)))
    acc = np.zeros((TT, C), dtype=np.float32)
    for r in res.results:
        acc += np.asarray(r["out"], dtype=np.float32)
    return acc.reshape(B, T, C)



# revision 8
# speedup vs baseline: 3.1122x; 3.1122x over previous
"""Tensor-parallel causal multi-head attention for 8 TRN2 NeuronCores.

Problem: B=2, T=2048, HIDDEN=2048, 16 heads x 128 head_dim, causal, RoPE.
Sharding: 2 heads per core (tensor parallel). Each core computes its QKV
projections, RoPE, causal attention, and a partial output projection over
its 256 hidden features; the host sums the 8 partial outputs.

v2 layout/engine changes vs v1:
  - V computed directly in [t, d] via xc-stationary matmuls (no V^T pass,
    no PE transposes or per-block copies for V).
  - RoPE entirely in bf16 with a host-side half-major head-dim permutation
    (even dims then odd dims) so every DVE operand is packed 16-bit.
  - Q^T/K^T transpose-block copies on DVE; PSUM->SBUF staging split between
    Act and DVE to balance engine load.
  - Causal trim: diagonal k-tiles compute scores/exp/PV only on the valid
    query range (memset of the invalid ptile region keeps masks NaN-safe).

Layouts (per core):
  xt    [2048 c, 4096 t]  bf16   (x transposed; contraction dim on partitions)
  wqt/wkt [2048 c, 256 d] bf16   (head-slice, transposed, half-major permuted)
  wvt   [2048 c, 256 d]  bf16   (head-slice of wv, transposed, unpermuted)
  wot   [256 c, 2048 d]  bf16   (per-core row-slice of wo.T)
  cos2/sin2 [2048 t, 128] bf16  (freqs duplicated across the 2 local heads)
  out   [4096 t, 2048 d]  bf16  partial output (host sums over cores)
"""

import numpy as np
import ml_dtypes
from contextlib import ExitStack

import concourse.bass as bass
import concourse.mybir as mybir
import concourse.tile as tile
from concourse import bacc
from concourse.bass_utils import run_bass_kernel_spmd
from concourse.masks import make_identity

F32 = mybir.dt.float32
BF16 = mybir.dt.bfloat16

NCORES = 8
B, T, C = 2, 2048, 2048
TT = B * T              # 4096 flattened rows
NH, D = 16, 128         # global heads, head dim
HL = NH // NCORES       # 2 local heads
DH = HL * D             # 256 local head features
NE = 8                  # t-eighths of 512 rows
ET = TT // NE           # 512 rows per eighth
CT = C // 128           # 16 contraction tiles
SCALE = 1.0 / float(np.sqrt(D))

_CACHE: dict = {}


def _build(T=T, B=B, num_devices=NCORES, repeat=1, small_out=False,
           ablate=()):
    TT = B * T
    NE = TT // 512
    ET = 512
    nc = bacc.Bacc("TRN2", target_bir_lowering=False, debug=False,
                   num_devices=num_devices)
    xt = nc.dram_tensor("xt", [C, TT], BF16, kind="ExternalInput").ap()
    wqt = nc.dram_tensor("wqt", [C, DH], BF16, kind="ExternalInput").ap()
    wkt = nc.dram_tensor("wkt", [C, DH], BF16, kind="ExternalInput").ap()
    wvt = nc.dram_tensor("wvt", [C, DH], BF16, kind="ExternalInput").ap()
    wot = nc.dram_tensor("wot", [DH, C], BF16, kind="ExternalInput").ap()
    cos2 = nc.dram_tensor("cos2", [T, 2 * (D // 2)], BF16, kind="ExternalInput").ap()
    sin2 = nc.dram_tensor("sin2", [T, 2 * (D // 2)], BF16, kind="ExternalInput").ap()
    _odt = F32 if "f32out" in ablate else BF16
    out = nc.dram_tensor("out", [128 if small_out else TT, C], _odt,
                         kind="ExternalOutput").ap()

    with ExitStack() as ctx:
        tc = ctx.enter_context(tile.TileContext(nc))
        # ---- persistent tiles -------------------------------------------
        gp = ctx.enter_context(tc.tile_pool(name="glob", bufs=1))
        # wqk packs [wq_c | wk_c] per c-tile so Q and K come from ONE
        # N=512 matmul (one PSUM accumulation group per bank).
        wqk_sb = gp.tile([128, CT * 2 * DH], BF16)   # [128, 8192]
        wv_sb = gp.tile([128, CT * DH], BF16)
        wo_sb = gp.tile([128, HL * C], BF16)    # [128, 4096]
        qk_view = wqk_sb[:].rearrange("p (k d) -> p k d", d=2 * DH)
        nc.sync.dma_start(qk_view[:, :, 0:DH],
                          wqt.rearrange("(k p) d -> p k d", p=128))
        nc.sync.dma_start(qk_view[:, :, DH:2 * DH],
                          wkt.rearrange("(k p) d -> p k d", p=128))
        for dst, src_ap, nd in ((wv_sb, wvt, DH), (wo_sb, wot, C)):
            nc.sync.dma_start(
                dst[:].rearrange("p (k d) -> p k d", d=nd),
                src_ap.rearrange("(k p) d -> p k d", p=128))

        v_all = gp.tile([128, (TT // 128) * DH], BF16)   # [128, 8192]
        qT = [gp.tile([128, TT], BF16, tag=f"qT{h}", name=f"qT{h}") for h in range(HL)]
        kT = [gp.tile([128, TT], BF16, tag=f"kT{h}", name=f"kT{h}") for h in range(HL)]

        ident = gp.tile([128, 128], BF16)
        make_identity(nc, ident[:])
        ones_col = gp.tile([128, 1], BF16)
        nc.vector.memset(ones_col[:], 1.0)
        ones_row = gp.tile([1, 128], F32)
        nc.vector.memset(ones_row[:], 1.0)

        # static causal masks for the 4 diagonal block offsets (f32-exact
        # iota, stored bf16 0/1): mask_k keeps [x, y] iff x <= y - 128k
        pairmasks = []
        mtmp = gp.tile([128, 512], F32)
        for m in range(2):
            pm = gp.tile([128, 1024], BF16, tag=f"pmask{m}", name=f"pmask{m}")
            for half in range(2):
                k = 2 * m + half
                nc.vector.memset(mtmp[:], 1.0)
                nc.gpsimd.affine_select(
                    out=mtmp[:], in_=mtmp[:],
                    compare_op=mybir.AluOpType.is_ge, fill=0.0,
                    base=-128 * k, pattern=[[1, 512]], channel_multiplier=-1,
                )
                nc.vector.tensor_copy(pm[:, half * 512:(half + 1) * 512], mtmp[:])
            pairmasks.append(pm)

        # ---- SBUF pools shared by both phases ---------------------------
        xp = ctx.enter_context(tc.tile_pool(name="xin", bufs=20))
        tp = ctx.enter_context(tc.tile_pool(name="trig", bufs=2))
        sp = ctx.enter_context(tc.tile_pool(name="stage", bufs=3))
        rp = ctx.enter_context(tc.tile_pool(name="rtmp", bufs=2))
        ptp = ctx.enter_context(tc.tile_pool(name="ptile", bufs=12))
        atp = ctx.enter_context(tc.tile_pool(name="attnT", bufs=6))
        rdp = ctx.enter_context(tc.tile_pool(name="rden", bufs=2))
        osp = ctx.enter_context(tc.tile_pool(name="ost", bufs=4))

        for _rep in range(repeat):
         # ---- phase 1: QKV projections + RoPE + Q/K transposes ----------
         with ExitStack() as p1:
            pqk = p1.enter_context(tc.tile_pool(name="pqk", bufs=4, space="PSUM"))
            pvp = p1.enter_context(tc.tile_pool(name="pv", bufs=2, space="PSUM"))
            pt = p1.enter_context(tc.tile_pool(name="ptr", bufs=2, space="PSUM"))

            for e in range(NE):
                t0 = e * ET  # global row offset of this eighth
                # per-eighth trig tiles [128, 4 x 128] (tt-major), bf16
                ct_sb = tp.tile([128, 4 * 128], BF16, tag="cos")
                st_sb = tp.tile([128, 4 * 128], BF16, tag="sin")
                trow = (t0 % T)
                nc.sync.dma_start(
                    ct_sb[:].rearrange("p (tt d) -> p tt d", d=128),
                    cos2[trow:trow + ET, :].rearrange("(tt p) d -> p tt d", p=128))
                nc.sync.dma_start(
                    st_sb[:].rearrange("p (tt d) -> p tt d", d=128),
                    sin2[trow:trow + ET, :].rearrange("(tt p) d -> p tt d", p=128))

                pQK = [pqk.tile([128, 512], F32, tag="pqk", name=f"pQK{_}")
                       for _ in range(4)]

                xcs = []
                for c in range(CT):
                    xc = xp.tile([128, ET], BF16, tag="xc")
                    dma_eng = nc.sync if c % 2 == 0 else nc.scalar
                    dma_eng.dma_start(
                        xc[:], xt[c * 128:(c + 1) * 128, t0:t0 + ET])
                    xcs.append(xc)
                    st = (c == 0)
                    sp_ = (c == CT - 1)
                    for tt in range(4):
                        nc.tensor.matmul(
                            pQK[tt][:], xc[:, tt * 128:(tt + 1) * 128],
                            wqk_sb[:, c * 2 * DH:(c + 1) * 2 * DH],
                            start=st, stop=sp_)

                # V [t, d] straight into v_all (tt-major chains re-reading
                # the resident xcs; 2 PSUM banks suffice)
                for tt in range(4):
                    g = (t0 // 128) + tt
                    pV = pvp.tile([128, 256], F32, tag="pv")
                    for c in range(CT):
                        nc.tensor.matmul(
                            pV[:], xcs[c][:, tt * 128:(tt + 1) * 128],
                            wv_sb[:, c * DH:(c + 1) * DH],
                            start=(c == 0), stop=(c == CT - 1))
                    nc.scalar.copy(v_all[:, g * DH:(g + 1) * DH], pV[:])

                # Q/K staged bf16 for RoPE (qs on DVE, ks on Act)
                qs = sp.tile([128, 4 * DH], BF16, tag="qs")
                ks = sp.tile([128, 4 * DH], BF16, tag="ks")
                for tt in range(4):
                    nc.vector.tensor_copy(qs[:, tt * DH:(tt + 1) * DH],
                                          pQK[tt][:, 0:256])
                    nc.scalar.copy(ks[:, tt * DH:(tt + 1) * DH],
                                   pQK[tt][:, 256:512])

                # RoPE in [t, d] layout, half-major head dim (E|O per head):
                #   out_E = E*c - O*s ; out_O = E*s + O*c
                # All operands bf16 + packed 64-wide runs -> DVE fast modes.
                qr = rp.tile([128, 4 * DH], BF16, tag="qr")
                kr = rp.tile([128, 4 * DH], BF16, tag="kr")
                tm1 = rp.tile([128, 4 * DH], BF16, tag="tm1")
                tm2 = rp.tile([128, 4 * DH], BF16, tag="tm2")
                cv = ct_sb[:].rearrange("p (tt h j) -> p tt h j", tt=4, h=HL)
                sv = st_sb[:].rearrange("p (tt h j) -> p tt h j", tt=4, h=HL)
                for src, dst in ((qs, qr), (ks, kr)):
                    s5 = src[:].rearrange(
                        "p (tt h half j) -> p tt h half j", tt=4, h=HL, half=2)
                    d5 = dst[:].rearrange(
                        "p (tt h half j) -> p tt h half j", tt=4, h=HL, half=2)
                    t5a = tm1[:].rearrange(
                        "p (tt h half j) -> p tt h half j", tt=4, h=HL, half=2)
                    t5b = tm2[:].rearrange(
                        "p (tt h half j) -> p tt h half j", tt=4, h=HL, half=2)
                    E, O = s5[:, :, :, 0, :], s5[:, :, :, 1, :]
                    nc.vector.tensor_mul(t5a[:, :, :, 0, :], E, cv)
                    nc.vector.tensor_mul(t5b[:, :, :, 0, :], O, sv)
                    nc.vector.tensor_sub(d5[:, :, :, 0, :],
                                         t5a[:, :, :, 0, :], t5b[:, :, :, 0, :])
                    nc.vector.tensor_mul(t5a[:, :, :, 1, :], E, sv)
                    nc.vector.tensor_mul(t5b[:, :, :, 1, :], O, cv)
                    nc.vector.tensor_add(d5[:, :, :, 1, :],
                                         t5a[:, :, :, 1, :], t5b[:, :, :, 1, :])

                # transpose Q/K blocks [128t, 128d] -> [128d, 128t]
                for src, dstl in ((qr, qT), (kr, kT)):
                    for tt in range(4):
                        for h in range(HL):
                            pb = pt.tile([128, 128], BF16, tag="ptr")
                            nc.tensor.transpose(
                                pb[:], src[:, tt * DH + h * 128: tt * DH + (h + 1) * 128],
                                ident[:])
                            nc.vector.tensor_copy(
                                dstl[h][:, t0 + tt * 128: t0 + (tt + 1) * 128],
                                pb[:])

         # ---- phase 2: attention + output projection ---------------------
         with ExitStack() as p2:
             psw = p2.enter_context(tc.tile_pool(name="psw", bufs=2, space="PSUM"))
             pso = p2.enter_context(tc.tile_pool(name="pso", bufs=2, space="PSUM"))
             psa = p2.enter_context(tc.tile_pool(name="psa", bufs=1, space="PSUM"))
             psd = p2.enter_context(tc.tile_pool(name="psd", bufs=1, space="PSUM"))

             for b in range(B):
                 for j in range(T // 512):   # q-chunks of 512 within the batch
                     q0 = b * T + j * 512
                     nkt = 4 * j + 4
                     attnT = []
                     for h in range(HL):
                         pA = psa.tile([128, 512], F32, tag="psa")
                         pDen = psd.tile([1, 512], F32, tag="psd")
                         npair = nkt // 2
                         for p_ in range(npair):
                             diag = (2 * p_ >= 4 * j)
                             pS = psw.tile([128, 1024], F32, tag="psw")
                             ptile = ptp.tile([128, 1024], BF16, tag="ptile")
                             for half in range(2):
                                 i = 2 * p_ + half
                                 g = b * (T // 128) + i
                                 r = i - 4 * j
                                 lo = 128 * r if (diag and r > 0) else 0
                                 nc.tensor.matmul(
                                     pS[:, half * 512 + lo:half * 512 + 512],
                                     kT[h][:, g * 128:(g + 1) * 128],
                                     qT[h][:, q0 + lo:q0 + 512],
                                     start=True, stop=True)
                             if diag:
                                 # per-half exp on the valid range; memset the
                                 # rest so mask-mult stays NaN-free
                                 for half in range(2):
                                     r = 2 * p_ + half - 4 * j
                                     lo = 128 * r
                                     if lo > 0:
                                         nc.vector.memset(
                                             ptile[:, half * 512:half * 512 + lo], 0.0)
                                     nc.scalar.activation(
                                         ptile[:, half * 512 + lo:half * 512 + 512],
                                         pS[:, half * 512 + lo:half * 512 + 512],
                                         mybir.ActivationFunctionType.Exp,
                                         scale=SCALE)
                                 nc.vector.tensor_mul(
                                     ptile[:], ptile[:], pairmasks[p_ - 2 * j][:])
                             else:
                                 nc.scalar.activation(
                                     ptile[:], pS[:],
                                     mybir.ActivationFunctionType.Exp,
                                     scale=SCALE)
                             for half in range(2):
                                 i = 2 * p_ + half
                                 g = b * (T // 128) + i
                                 r = i - 4 * j
                                 lo = 128 * r if (diag and r > 0) else 0
                                 nc.tensor.matmul(
                                     pA[:, lo:512],
                                     v_all[:, g * DH + h * 128: g * DH + (h + 1) * 128],
                                     ptile[:, half * 512 + lo:half * 512 + 512],
                                     start=(i == 0), stop=(i == nkt - 1),
                                     skip_group_check=True)
                                 # denominator straight off each ptile half
                                 # (memset/masked zeros contribute nothing)
                                 nc.tensor.matmul(
                                     pDen[:], ones_col[:],
                                     ptile[:, half * 512:half * 512 + 512],
                                     start=(i == 0), stop=(i == nkt - 1))
                         rden = rdp.tile([1, 512], F32, tag="rden")
                         nc.vector.reciprocal(rden[:], pDen[:])
                         pB = pso.tile([128, 512], F32, tag="pso")
                         nc.tensor.matmul(pB[:], ones_row[:], rden[:],
                                          start=True, stop=True)
                         bc = rdp.tile([128, 512], F32, tag="bc")
                         nc.scalar.copy(bc[:], pB[:])
                         aT = atp.tile([128, 512], BF16, tag=f"aT{h}")
                         nc.vector.tensor_mul(aT[:], pA[:], bc[:])
                         attnT.append(aT)

                     for tt in range(4):
                         r0 = q0 + tt * 128
                         ost = osp.tile([128, C], _odt, tag="ost")
                         for oc in range(4):
                             pO = pso.tile([128, 512], F32, tag="pso")
                             for h in range(HL):
                                 nc.tensor.matmul(
                                     pO[:], attnT[h][:, tt * 128:(tt + 1) * 128],
                                     wo_sb[:, h * C + oc * 512: h * C + oc * 512 + 512],
                                     start=(h == 0), stop=(h == HL - 1))
                             if oc == 0:
                                 nc.scalar.copy(ost[:, 0:512], pO[:])
                             else:
                                 nc.vector.tensor_copy(
                                     ost[:, oc * 512:(oc + 1) * 512], pO[:])
                         if small_out:
                             nc.scalar.dma_start(out[0:128, :], ost[:])
                         else:
                             nc.scalar.dma_start(out[r0:r0 + 128, :], ost[:])

    nc.compile()
    return nc


def _get_nc():
    if "nc" not in _CACHE:
        _CACHE["nc"] = _build()
    return _CACHE["nc"]


# half-major permutation of a head's 128 dims: [0,2,...,126, 1,3,...,127]
_PERM = np.concatenate([np.arange(0, D, 2), np.arange(1, D, 2)])


def _permute_heads(w_slice):
    # w_slice: [DH, C] rows = local head dims (HL heads x 128)
    out = np.empty_like(w_slice)
    for l in range(HL):
        out[l * D:(l + 1) * D] = w_slice[l * D + _PERM]
    return out


def kernel(x, wq, wk, wv, wo, freqs_cos, freqs_sin, mask=None, **_unused):
    bf = ml_dtypes.bfloat16
    nc = _get_nc()

    x = np.asarray(x, dtype=np.float32)
    xt = np.ascontiguousarray(x.reshape(TT, C).T).astype(bf)
    cos2 = np.ascontiguousarray(
        np.tile(np.asarray(freqs_cos, np.float32), (1, HL))).astype(bf)
    sin2 = np.ascontiguousarray(
        np.tile(np.asarray(freqs_sin, np.float32), (1, HL))).astype(bf)

    in_maps = []
    for i in range(NCORES):
        sl = slice(DH * i, DH * (i + 1))
        wq_s = _permute_heads(np.asarray(wq, np.float32)[sl, :])
        wk_s = _permute_heads(np.asarray(wk, np.float32)[sl, :])
        in_maps.append({
            "xt": xt,
            "wqt": np.ascontiguousarray(wq_s.T).astype(bf),
            "wkt": np.ascontiguousarray(wk_s.T).astype(bf),
            "wvt": np.ascontiguousarray(np.asarray(wv, np.float32)[sl, :].T).astype(bf),
            "wot": np.ascontiguousarray(np.asarray(wo, np.float32)[:, sl].T).astype(bf),
            "cos2": cos2,
            "sin2": sin2,
        })

    res = run_bass_kernel_spmd(nc, in_maps, core_ids=list(range(NCORES)))
    acc = np.zeros((TT, C), dtype=np.float32)
    for r in res.results:
        acc += np.asarray(r["out"], dtype=np.float32)
    return acc.reshape(B, T, C)
